# revision 4
# baseline (speedup 1.0000x reference)
"""CausalMaskedLinear Trainium2 kernel.

y = x @ (W * mask).T + b, with mask a deterministic block-banded causal
pattern: output time-step block o (128 rows) attends to input blocks
j in [o-7, o] (TRI_BLOCK=8), 128 cols each.  Only 228 of the 1024
128x128 weight blocks are live.

Strategy: data-parallel over batch (8192/8 = 1024 rows per core),
weights/bias replicated.  Host packs x transposed ([in_feat, batch]) and
the live weight blocks transposed ([in, out] layout) so the device loop
is a pure stream of PSUM-accumulated matmuls:
    yT[o*128:, b] = sum_j WT_block(o,j).T @ xT_block(j)[:, b]   (+ bias)

Mixed precision: for each output block's band, the two OLDEST input
blocks (an even-aligned pair) are contracted in a single fp8-e4m3
DoubleRow matmul (2 blocks per PE pass, element-interleaved moving
operand so the PE streams 2 MACs/cell/cycle); the remaining blocks run
in fp16 (1 block per pass).  All weights are pre-scaled by 512 (power
of two, exact) so fp8 weight values sit in e4m3's normal range and both
precisions accumulate at the same scale in fp32 PSUM; the PSUM->SBUF
copy applies x(1/512) + bias in one dual-op vector instruction.
Max relative error ~1.4e-2 (vs 2e-2 gate), dominated by the fp8 pair.

Output is written fp16 (halves store traffic; adds ~2e-4 error),
restored to fp32 + untransposed on host.
"""

import numpy as np

NUM_TIME_STEPS = 32
IN_FEAT = 128
OUT_FEAT = 128
TRI_BLOCK = 8
BATCH = 8192
N_CORES = 8
BC = BATCH // N_CORES  # batch rows per core
NH = BC // 512         # 512-col PSUM pieces per output tile

IN_SIZE = NUM_TIME_STEPS * IN_FEAT
OUT_SIZE = NUM_TIME_STEPS * OUT_FEAT

WSCALE = 512.0         # power of two: weight pre-scale (exact to undo)
USE_FP8 = True         # one DoubleRow fp8 pass per band (2 blocks)
N_Q = 13               # even block-pairs 0..12 used by some band


def _band(o):
    return list(range(max(0, o - TRI_BLOCK + 1), o + 1))


def _pair(o):
    """Even-aligned block pair computed via fp8 DoubleRow, or None."""
    if not USE_FP8 or o == 0:
        return None
    lo = max(0, o - TRI_BLOCK + 1)
    p = lo if lo % 2 == 0 else lo + 1
    assert p + 1 <= o
    return p


def _f16_blocks(o):
    p = _pair(o)
    if p is None:
        return _band(o)
    return [j for j in _band(o) if j != p and j != p + 1]


# fp16 weight-block packing: groups of 4 consecutive o's, blocks (o, j)
# for j in _f16_blocks(o), o ascending, j ascending, contiguous per group.
_W16_GROUPS = []           # per group: list of (o, j)
_W16_BASE = {}             # o -> first block index within its group tile
for _g in range(NUM_TIME_STEPS // 4):
    blks = []
    for _o in range(4 * _g, 4 * _g + 4):
        _W16_BASE[_o] = len(blks)
        blks.extend((_o, _j) for _j in _f16_blocks(_o))
    _W16_GROUPS.append(blks)

_N_PAIR_O = NUM_TIME_STEPS - 1           # o = 1..31 each have one fp8 pair

_PROGRAM = None


def _build_program():
    import concourse.bacc as bacc
    import concourse.bass as bass
    import concourse.mybir as mybir
    import concourse.tile as tile

    f32 = mybir.dt.float32
    f16 = mybir.dt.float16
    f8 = mybir.dt.float8e4

    nc = bacc.Bacc("TRN2", target_bir_lowering=False, debug=False,
                   enable_asserts=False)

    xT_d = nc.dram_tensor("xT", [128, NUM_TIME_STEPS, BC], f16,
                          kind="ExternalInput")
    wt_d = nc.dram_tensor("wt", [128, sum(len(g) for g in _W16_GROUPS) * 128],
                          f16, kind="ExternalInput")
    bias_d = nc.dram_tensor("bias_t", [128, NUM_TIME_STEPS], f32,
                            kind="ExternalInput")
    if USE_FP8:
        # element-interleaved even pairs: x8[k, q, n, i] = fp8(x)[k, 2q+i, n]
        x8_d = nc.dram_tensor("x8", [128, N_Q, BC, 2], f8,
                              kind="ExternalInput")
        w8_d = nc.dram_tensor("w8", [128, 2 * _N_PAIR_O, 128], f8,
                              kind="ExternalInput")
    yT_d = nc.dram_tensor("yT", [NUM_TIME_STEPS, 128, BC], f16,
                          kind="ExternalOutput")

    DR = mybir.MatmulPerfMode.DoubleRow
    MULT = mybir.AluOpType.mult
    ADD = mybir.AluOpType.add
    INV = 1.0 / WSCALE

    with tile.TileContext(nc) as tc:
        with (
            tc.tile_pool(name="x0p", bufs=2) as x0p,
            tc.tile_pool(name="xp", bufs=15) as xp,
            tc.tile_pool(name="wp", bufs=len(_W16_GROUPS)) as wp,
            tc.tile_pool(name="x8p", bufs=N_Q) as x8p,
            tc.tile_pool(name="w8p", bufs=1) as w8p,
            tc.tile_pool(name="op", bufs=12) as op,
            tc.tile_pool(name="mp", bufs=2) as mp,
            tc.tile_pool(name="psp", bufs=8, space=bass.MemorySpace.PSUM) as psp,
        ):
            # ---- PE warm-up -------------------------------------------
            # HAM un-throttles the PE (1.2 -> 2.4 GHz) only after ~3.4us
            # of sustained activity; burn the head DMA latency on dummy
            # matmuls so the real stream hits 2.4 GHz as early as
            # possible.  memset on gpsimd (earliest-ready engine).
            warm = mp.tile([128, 512], f16, tag="warm")
            nc.gpsimd.memset(warm[:], 0.0)
            warm_ps = psp.tile([128, 512], f32, tag="ps")
            nc.tensor.matmul(warm_ps[:], warm[:, :128], warm[:],
                             start=True, stop=True)
            # fine-grained tail keeps the PE busy up to data arrival
            # without delaying the first real matmul by more than ~110ns
            for _ in range(10):
                nc.tensor.matmul(warm_ps[:, :128], warm[:, :128],
                                 warm[:, :128], start=True, stop=True)

            # ---- loads ------------------------------------------------
            # sync queue: x blocks 0 (split for earliest start) and 1,
            # then w16 groups 1..7, then the output stores.
            # scalar queue: everything else in compute-need order.
            bias_t = mp.tile([128, NUM_TIME_STEPS], f32, tag="bias")
            wg = [None] * len(_W16_GROUPS)
            x8q = [None] * N_Q
            w8t = None
            if USE_FP8:
                w8t = w8p.tile([128, 2 * _N_PAIR_O, 128], f8, tag="w8")

            x0t = x0p.tile([128, BC], f16, tag="x0")
            nc.sync.dma_start(x0t[:, :512], xT_d[:, 0, :512])
            nc.sync.dma_start(x0t[:, 512:], xT_d[:, 0, 512:])
            x1t = x0p.tile([128, BC], f16, tag="x0")
            nc.sync.dma_start(x1t[:], xT_d[:, 1, :])

            x16p = [None] * 16            # pair tiles for blocks (2a, 2a+1)

            def x16(j):
                if j == 0:
                    return x0t[:]
                if j == 1:
                    return x1t[:]
                return x16p[j // 2][:, j % 2, :]

            for g in range(1, len(_W16_GROUPS)):
                n = len(_W16_GROUPS[g])
                t = wp.tile([128, n * 128], f16, tag="w")
                off = sum(len(gg) for gg in _W16_GROUPS[:g]) * 128
                nc.sync.dma_start(t[:], wt_d[:, off:off + n * 128])
                wg[g] = t

            items = [("wg", 0, 0.0), ("bias", 0, 0.5)]
            if USE_FP8:
                items += [("x8", 0, 0.8), ("w8", 0, 1.0),
                          ("w8", 1, 7.5), ("w8", 2, 15.5)]
                items += [("x8", q, 2 * q + 5.5) for q in range(1, N_Q)]
            items += [("x16", a, float(2 * a)) for a in range(1, 16)]
            items.sort(key=lambda it: it[2])

            W8_SPLITS = [(0, 14), (14, 30), (30, 2 * _N_PAIR_O)]
            for kind, idx, _need in items:
                if kind == "wg":
                    n = len(_W16_GROUPS[idx])
                    t = wp.tile([128, n * 128], f16, tag="w")
                    off = sum(len(gg) for gg in _W16_GROUPS[:idx]) * 128
                    nc.scalar.dma_start(t[:], wt_d[:, off:off + n * 128])
                    wg[idx] = t
                elif kind == "bias":
                    nc.scalar.dma_start(bias_t[:], bias_d[:])
                elif kind == "x16":
                    t = xp.tile([128, 2, BC], f16, tag="x")
                    nc.scalar.dma_start(
                        t[:], xT_d[:, 2 * idx:2 * idx + 2, :])
                    x16p[idx] = t
                elif kind == "x8":
                    t = x8p.tile([128, BC, 2], f8, tag="x8")
                    nc.scalar.dma_start(t[:], x8_d[:, idx, :, :])
                    x8q[idx] = t
                elif kind == "w8":
                    a, b = W8_SPLITS[idx]
                    nc.scalar.dma_start(w8t[:, a:b, :], w8_d[:, a:b, :])

            # ---- compute ----------------------------------------------
            for o in range(NUM_TIME_STEPS):
                f16js = _f16_blocks(o)
                p = _pair(o)
                base = _W16_BASE[o]
                wgt = wg[o // 4]
                out_t = op.tile([128, BC], f16, tag="o")
                for h in range(NH):
                    hs = slice(h * 512, (h + 1) * 512)
                    ps = psp.tile([128, 512], f32, tag="ps")
                    n = (1 if p is not None else 0) + len(f16js)
                    k = 0
                    if p is not None:
                        mov = x8q[p // 2][:, hs, :].transpose([0, 2, 1])
                        nc.tensor.matmul(
                            ps[:], w8t[:, 2 * (o - 1):2 * o, :], mov,
                            start=True, stop=(n == 1), perf_mode=DR)
                        k = 1
                    for i, j in enumerate(f16js):
                        nc.tensor.matmul(
                            ps[:],
                            wgt[:, (base + i) * 128:(base + i + 1) * 128],
                            x16(j)[:, hs],
                            start=(k == 0), stop=(k == n - 1))
                        k += 1
                    # PSUM -> SBUF: out = ps/WSCALE + bias  (one dual-op)
                    if o == NUM_TIME_STEPS - 1 and h == NH - 1:
                        # split the last piece so the final store starts
                        # ~0.4us earlier
                        for c in range(2):
                            cs = slice(h * 512 + c * 256,
                                       h * 512 + (c + 1) * 256)
                            nc.vector.tensor_scalar(
                                out=out_t[:, cs],
                                in0=ps[:, c * 256:(c + 1) * 256],
                                scalar1=INV, scalar2=bias_t[:, o:o + 1],
                                op0=MULT, op1=ADD)
                            nc.sync.dma_start(yT_d[o][:, cs], out_t[:, cs])
                    else:
                        nc.vector.tensor_scalar(
                            out=out_t[:, hs], in0=ps[:],
                            scalar1=INV, scalar2=bias_t[:, o:o + 1],
                            op0=MULT, op1=ADD)
                        if o == NUM_TIME_STEPS - 1:
                            nc.sync.dma_start(yT_d[o][:, hs], out_t[:, hs])
                if o < NUM_TIME_STEPS - 1:
                    nc.sync.dma_start(yT_d[o], out_t[:])

    nc.compile()
    return nc


def _get_program():
    global _PROGRAM
    if _PROGRAM is None:
        _PROGRAM = _build_program()
    return _PROGRAM


def _pack_inputs(x, weight, bias, mask):
    import ml_dtypes

    F8 = ml_dtypes.float8_e4m3  # TRN flavor (max 240); values stay < 16

    x = np.asarray(x, dtype=np.float32)
    weight = np.asarray(weight, dtype=np.float32)
    bias = np.asarray(bias, dtype=np.float32)
    mask = np.asarray(mask)

    ws = (weight * mask * WSCALE).astype(np.float32)

    n16 = sum(len(g) for g in _W16_GROUPS)
    wt16 = np.empty((128, n16 * 128), dtype=np.float16)
    k = 0
    for g in _W16_GROUPS:
        for (o, j) in g:
            blk = ws[o * 128:(o + 1) * 128, j * 128:(j + 1) * 128]
            wt16[:, k * 128:(k + 1) * 128] = blk.T
            k += 1

    if USE_FP8:
        w8 = np.empty((128, 2 * _N_PAIR_O, 128), dtype=F8)
        for o in range(1, NUM_TIME_STEPS):
            p = _pair(o)
            for i in (0, 1):
                blk = ws[o * 128:(o + 1) * 128,
                         (p + i) * 128:(p + i + 1) * 128]
                w8[:, 2 * (o - 1) + i, :] = blk.T.astype(F8)

    bias_t = np.ascontiguousarray(bias.reshape(NUM_TIME_STEPS, 128).T)

    x16 = x.astype(np.float16)
    in_maps = []
    for c in range(N_CORES):
        xc = x16[c * BC:(c + 1) * BC]  # [BC, 4096]
        xTc = np.ascontiguousarray(
            xc.reshape(BC, NUM_TIME_STEPS, 128).transpose(2, 1, 0))
        m = {"xT": xTc, "wt": wt16, "bias_t": bias_t}
        if USE_FP8:
            xTc8 = xTc.astype(F8)          # [128, 32, BC]
            x8i = np.empty((128, N_Q, BC, 2), dtype=F8)
            for q in range(N_Q):
                x8i[:, q, :, 0] = xTc8[:, 2 * q, :]
                x8i[:, q, :, 1] = xTc8[:, 2 * q + 1, :]
            m["x8"] = x8i
            m["w8"] = w8
        in_maps.append(m)
    return in_maps


def _run(inputs, trace=False):
    from concourse.bass_utils import run_bass_kernel_spmd

    nc = _get_program()
    in_maps = _pack_inputs(**inputs)
    res = run_bass_kernel_spmd(nc, in_maps, list(range(N_CORES)), trace=trace)

    y = np.empty((BATCH, OUT_SIZE), dtype=np.float32)
    for c in range(N_CORES):
        yTc = res.results[c]["yT"].reshape(OUT_SIZE, BC)
        y[c * BC:(c + 1) * BC] = yTc.T.astype(np.float32)
    return y, res


def kernel(x, weight, bias, mask):
    y, _ = _run({"x": x, "weight": weight, "bias": bias, "mask": mask})
    return y


# revision 8
# speedup vs baseline: 1.2324x; 1.2324x over previous
"""CausalMaskedLinear Trainium2 kernel.

y = x @ (W * mask).T + b, with mask a deterministic block-banded causal
pattern: output time-step block o (128 rows) attends to input blocks
j in [o-7, o] (TRI_BLOCK=8), 128 cols each.  Only 228 of the 1024
128x128 weight blocks are live.

Strategy: data-parallel over batch (8192/8 = 1024 rows per core),
weights/bias replicated.  Host packs x transposed ([in_feat, batch]) and
the live weight blocks transposed ([in, out] layout) so the device loop
is a pure stream of PSUM-accumulated matmuls:
    yT[o*128:, b] = sum_j WT_block(o,j).T @ xT_block(j)[:, b]   (+ bias)

Mixed precision: for each output block's band, the two OLDEST input
blocks (an even-aligned pair) are contracted in a single fp8-e4m3
DoubleRow matmul (2 blocks per PE pass, element-interleaved moving
operand so the PE streams 2 MACs/cell/cycle); the remaining blocks run
in fp16 (1 block per pass).  All weights are pre-scaled by 512 (power
of two, exact) so fp8 weight values sit in e4m3's normal range and both
precisions accumulate at the same scale in fp32 PSUM; the PSUM->SBUF
copy applies x(1/512) + bias in one dual-op vector instruction.
Max relative error ~1.4e-2 (vs 2e-2 gate), dominated by the fp8 pair.

Output is written fp16 (halves store traffic; adds ~2e-4 error),
restored to fp32 + untransposed on host.
"""

import numpy as np

NUM_TIME_STEPS = 32
IN_FEAT = 128
OUT_FEAT = 128
TRI_BLOCK = 8
BATCH = 8192
N_CORES = 8
BC = BATCH // N_CORES  # batch rows per core
NH = BC // 512         # 512-col PSUM pieces per output tile

IN_SIZE = NUM_TIME_STEPS * IN_FEAT
OUT_SIZE = NUM_TIME_STEPS * OUT_FEAT

WSCALE = 512.0         # power of two: weight pre-scale (exact to undo)
USE_FP8 = True         # one DoubleRow fp8 pass per band (2 blocks)
N_Q = 13               # even block-pairs 0..12 used by some band


def _band(o):
    return list(range(max(0, o - TRI_BLOCK + 1), o + 1))


def _pair(o):
    """Even-aligned block pair computed via fp8 DoubleRow, or None."""
    if not USE_FP8 or o == 0:
        return None
    lo = max(0, o - TRI_BLOCK + 1)
    p = lo if lo % 2 == 0 else lo + 1
    assert p + 1 <= o
    return p


def _f16_blocks(o):
    p = _pair(o)
    if p is None:
        return _band(o)
    return [j for j in _band(o) if j != p and j != p + 1]


# fp16 weight-block packing: groups of 4 consecutive o's, blocks (o, j)
# for j in _f16_blocks(o), o ascending, j ascending, contiguous per group.
_W16_GROUPS = []           # per group: list of (o, j)
_W16_BASE = {}             # o -> first block index within its group tile
for _g in range(NUM_TIME_STEPS // 4):
    blks = []
    for _o in range(4 * _g, 4 * _g + 4):
        _W16_BASE[_o] = len(blks)
        blks.extend((_o, _j) for _j in _f16_blocks(_o))
    _W16_GROUPS.append(blks)

_N_PAIR_O = NUM_TIME_STEPS - 1           # o = 1..31 each have one fp8 pair

_PROGRAM = None


def _build_program():
    import concourse.bacc as bacc
    import concourse.bass as bass
    import concourse.mybir as mybir
    import concourse.tile as tile

    f32 = mybir.dt.float32
    f16 = mybir.dt.float16
    f8 = mybir.dt.float8e4

    nc = bacc.Bacc("TRN2", target_bir_lowering=False, debug=False,
                   enable_asserts=False)

    xT_d = nc.dram_tensor("xT", [128, NUM_TIME_STEPS, BC], f16,
                          kind="ExternalInput")
    wt_d = nc.dram_tensor("wt", [128, sum(len(g) for g in _W16_GROUPS) * 128],
                          f16, kind="ExternalInput")
    bias_d = nc.dram_tensor("bias_t", [128, NUM_TIME_STEPS], f32,
                            kind="ExternalInput")
    if USE_FP8:
        # element-interleaved even pairs: x8[k, q, n, i] = fp8(x)[k, 2q+i, n]
        x8_d = nc.dram_tensor("x8", [128, N_Q, BC, 2], f8,
                              kind="ExternalInput")
        w8_d = nc.dram_tensor("w8", [128, 2 * _N_PAIR_O, 128], f8,
                              kind="ExternalInput")
    yT_d = nc.dram_tensor("yT", [NUM_TIME_STEPS, 128, BC], f16,
                          kind="ExternalOutput")

    DR = mybir.MatmulPerfMode.DoubleRow
    MULT = mybir.AluOpType.mult
    ADD = mybir.AluOpType.add
    INV = 1.0 / WSCALE

    with tile.TileContext(nc) as tc:
        with (
            tc.tile_pool(name="w0p", bufs=1) as w0p,
            tc.tile_pool(name="x0p", bufs=2) as x0p,
            tc.tile_pool(name="xp", bufs=15) as xp,
            tc.tile_pool(name="wp", bufs=len(_W16_GROUPS) - 1) as wp,
            tc.tile_pool(name="x8p", bufs=N_Q) as x8p,
            tc.tile_pool(name="w8p", bufs=1) as w8p,
            tc.tile_pool(name="op", bufs=12) as op,
            tc.tile_pool(name="mp", bufs=2) as mp,
            tc.tile_pool(name="psp", bufs=8, space=bass.MemorySpace.PSUM) as psp,
        ):
            # ---- PE warm-up -------------------------------------------
            # HAM un-throttles the PE (1.2 -> 2.4 GHz) only after ~3.4us
            # of sustained activity; burn the head DMA latency on dummy
            # matmuls so the real stream hits 2.4 GHz as early as
            # possible.  memset on gpsimd (earliest-ready engine).
            warm = mp.tile([128, 512], f16, tag="warm")
            nc.gpsimd.memset(warm[:], 0.0)
            warm_ps = psp.tile([128, 512], f32, tag="ps")
            nc.tensor.matmul(warm_ps[:], warm[:, :128], warm[:],
                             start=True, stop=True)
            # fine-grained tail keeps the PE busy up to data arrival
            # without delaying the first real matmul by more than ~110ns
            for _ in range(10):
                nc.tensor.matmul(warm_ps[:, :128], warm[:, :128],
                                 warm[:, :128], start=True, stop=True)

            # ---- loads ------------------------------------------------
            # sync queue: x blocks 0 (split for earliest start) and 1,
            # then w16 groups 1..7, then the output stores.
            # scalar queue: everything else in compute-need order.
            bias_t = mp.tile([128, NUM_TIME_STEPS], f32, tag="bias")
            wg = [None] * len(_W16_GROUPS)
            x8q = [None] * N_Q
            w8t = None
            if USE_FP8:
                w8t = w8p.tile([128, 2 * _N_PAIR_O, 128], f8, tag="w8")

            # wg0 (the o=0..3 fp16 weights) FIRST: the whole pipeline
            # starts with it, and the scheduler orders DMAs by tile
            # creation, so create + issue it before everything else.
            n0 = len(_W16_GROUPS[0])
            wg0t = w0p.tile([128, n0 * 128], f16, tag="w0")
            nc.scalar.dma_start(wg0t[:], wt_d[:, :n0 * 128])
            wg[0] = wg0t

            x0t = x0p.tile([128, BC], f16, tag="x0")
            nc.sync.dma_start(x0t[:, :512], xT_d[:, 0, :512])
            nc.sync.dma_start(x0t[:, 512:], xT_d[:, 0, 512:])
            x1t = x0p.tile([128, BC], f16, tag="x0")
            nc.sync.dma_start(x1t[:], xT_d[:, 1, :])

            x16p = [None] * 16            # pair tiles for blocks (2a, 2a+1)

            def x16(j):
                if j == 0:
                    return x0t[:]
                if j == 1:
                    return x1t[:]
                return x16p[j // 2][:, j % 2, :]

            for g in range(1, len(_W16_GROUPS)):
                n = len(_W16_GROUPS[g])
                t = wp.tile([128, n * 128], f16, tag="w")
                off = sum(len(gg) for gg in _W16_GROUPS[:g]) * 128
                nc.sync.dma_start(t[:], wt_d[:, off:off + n * 128])
                wg[g] = t

            items = [("bias", 0, 0.5)]
            if USE_FP8:
                items += [("x8", 0, 0.8), ("w8", 0, 1.0),
                          ("w8", 1, 7.5), ("w8", 2, 15.5)]
                items += [("x8", q, 2 * q + 5.5) for q in range(1, N_Q)]
            items += [("x16", a, float(2 * a)) for a in range(1, 16)]
            items.sort(key=lambda it: it[2])

            W8_SPLITS = [(0, 14), (14, 30), (30, 2 * _N_PAIR_O)]
            for kind, idx, _need in items:
                if kind == "wg":
                    n = len(_W16_GROUPS[idx])
                    t = wp.tile([128, n * 128], f16, tag="w")
                    off = sum(len(gg) for gg in _W16_GROUPS[:idx]) * 128
                    nc.scalar.dma_start(t[:], wt_d[:, off:off + n * 128])
                    wg[idx] = t
                elif kind == "bias":
                    nc.scalar.dma_start(bias_t[:], bias_d[:])
                elif kind == "x16":
                    t = xp.tile([128, 2, BC], f16, tag="x")
                    nc.scalar.dma_start(
                        t[:], xT_d[:, 2 * idx:2 * idx + 2, :])
                    x16p[idx] = t
                elif kind == "x8":
                    t = x8p.tile([128, BC, 2], f8, tag="x8")
                    nc.scalar.dma_start(t[:], x8_d[:, idx, :, :])
                    x8q[idx] = t
                elif kind == "w8":
                    a, b = W8_SPLITS[idx]
                    nc.scalar.dma_start(w8t[:, a:b, :], w8_d[:, a:b, :])

            # ---- compute ----------------------------------------------
            for o in range(NUM_TIME_STEPS):
                f16js = _f16_blocks(o)
                p = _pair(o)
                base = _W16_BASE[o]
                wgt = wg[o // 4]
                out_t = op.tile([128, BC], f16, tag="o")
                for h in range(NH):
                    hs = slice(h * 512, (h + 1) * 512)
                    ps = psp.tile([128, 512], f32, tag="ps")
                    # A DoubleRow matmul that OPENS an accumulation group
                    # runs at half rate (~408ns vs ~220); put the DR pass
                    # second whenever the band has fp16 blocks.
                    seq = []
                    if f16js:
                        seq.append(("f", 0))
                        if p is not None:
                            seq.append(("d", None))
                        seq += [("f", i) for i in range(1, len(f16js))]
                    else:
                        seq.append(("d", None))
                    for k, (kind, i) in enumerate(seq):
                        first, last = k == 0, k == len(seq) - 1
                        if kind == "d":
                            mov = x8q[p // 2][:, hs, :].transpose([0, 2, 1])
                            nc.tensor.matmul(
                                ps[:], w8t[:, 2 * (o - 1):2 * o, :], mov,
                                start=first, stop=last, perf_mode=DR)
                        else:
                            j = f16js[i]
                            nc.tensor.matmul(
                                ps[:],
                                wgt[:, (base + i) * 128:(base + i + 1) * 128],
                                x16(j)[:, hs],
                                start=first, stop=last)
                    # PSUM -> SBUF: out = ps/WSCALE + bias  (one dual-op)
                    if o == NUM_TIME_STEPS - 1 and h == NH - 1:
                        # split the last piece so the final store starts
                        # ~0.4us earlier
                        for c in range(2):
                            cs = slice(h * 512 + c * 256,
                                       h * 512 + (c + 1) * 256)
                            nc.vector.tensor_scalar(
                                out=out_t[:, cs],
                                in0=ps[:, c * 256:(c + 1) * 256],
                                scalar1=INV, scalar2=bias_t[:, o:o + 1],
                                op0=MULT, op1=ADD)
                            nc.sync.dma_start(yT_d[o][:, cs], out_t[:, cs])
                    else:
                        nc.vector.tensor_scalar(
                            out=out_t[:, hs], in0=ps[:],
                            scalar1=INV, scalar2=bias_t[:, o:o + 1],
                            op0=MULT, op1=ADD)
                        if o == NUM_TIME_STEPS - 1:
                            nc.sync.dma_start(yT_d[o][:, hs], out_t[:, hs])
                if o < NUM_TIME_STEPS - 1:
                    nc.sync.dma_start(yT_d[o], out_t[:])

    nc.compile()
    return nc


def _get_program():
    global _PROGRAM
    if _PROGRAM is None:
        _PROGRAM = _build_program()
    return _PROGRAM


def _pack_inputs(x, weight, bias, mask):
    import ml_dtypes

    F8 = ml_dtypes.float8_e4m3  # TRN flavor (max 240); values stay < 16

    x = np.asarray(x, dtype=np.float32)
    weight = np.asarray(weight, dtype=np.float32)
    bias = np.asarray(bias, dtype=np.float32)
    mask = np.asarray(mask)

    ws = (weight * mask * WSCALE).astype(np.float32)

    n16 = sum(len(g) for g in _W16_GROUPS)
    wt16 = np.empty((128, n16 * 128), dtype=np.float16)
    k = 0
    for g in _W16_GROUPS:
        for (o, j) in g:
            blk = ws[o * 128:(o + 1) * 128, j * 128:(j + 1) * 128]
            wt16[:, k * 128:(k + 1) * 128] = blk.T
            k += 1

    if USE_FP8:
        w8 = np.empty((128, 2 * _N_PAIR_O, 128), dtype=F8)
        for o in range(1, NUM_TIME_STEPS):
            p = _pair(o)
            for i in (0, 1):
                blk = ws[o * 128:(o + 1) * 128,
                         (p + i) * 128:(p + i + 1) * 128]
                w8[:, 2 * (o - 1) + i, :] = blk.T.astype(F8)

    bias_t = np.ascontiguousarray(bias.reshape(NUM_TIME_STEPS, 128).T)

    x16 = x.astype(np.float16)
    in_maps = []
    for c in range(N_CORES):
        xc = x16[c * BC:(c + 1) * BC]  # [BC, 4096]
        xTc = np.ascontiguousarray(
            xc.reshape(BC, NUM_TIME_STEPS, 128).transpose(2, 1, 0))
        m = {"xT": xTc, "wt": wt16, "bias_t": bias_t}
        if USE_FP8:
            xTc8 = xTc.astype(F8)          # [128, 32, BC]
            x8i = np.empty((128, N_Q, BC, 2), dtype=F8)
            for q in range(N_Q):
                x8i[:, q, :, 0] = xTc8[:, 2 * q, :]
                x8i[:, q, :, 1] = xTc8[:, 2 * q + 1, :]
            m["x8"] = x8i
            m["w8"] = w8
        in_maps.append(m)
    return in_maps


def _run(inputs, trace=False):
    from concourse.bass_utils import run_bass_kernel_spmd

    nc = _get_program()
    in_maps = _pack_inputs(**inputs)
    res = run_bass_kernel_spmd(nc, in_maps, list(range(N_CORES)), trace=trace)

    y = np.empty((BATCH, OUT_SIZE), dtype=np.float32)
    for c in range(N_CORES):
        yTc = res.results[c]["yT"].reshape(OUT_SIZE, BC)
        y[c * BC:(c + 1) * BC] = yTc.T.astype(np.float32)
    return y, res


def kernel(x, weight, bias, mask):
    y, _ = _run({"x": x, "weight": weight, "bias": bias, "mask": mask})
    return y
